# revision 40
# baseline (speedup 1.0000x reference)
"""GAT layer kernel for Trainium2, 8 NeuronCores.

Reference computation:
    X = node_features @ W            [N, DOUT]
    f0 = X @ v0 ; f1 = X @ v1       [N, 1]
    vals = sigmoid(f0 + f1.T) - 0.5
    alphas = softmax(where(graph != 0, vals, -inf), axis=1), masked to 0
    out = elu(alphas @ X)

Key identities / design:
  * softmax ratio: out_row = (sum_j m_ij w_ij X_j) / (sum_j m_ij w_ij) with
    w = exp(sigmoid(z)-0.5), z = f0_i + f1_j; the row-sum comes free as a
    ones-column in the matmul rhs, so the [N,N] attention matrix is never
    normalized at full width.
  * w*(z) = exp(sigmoid(z)-0.5) is itself an S-curve with exponential
    tails; a global affine of a logistic matches it to |err| <= 5.3e-4:
        w*(z) ~= A*sigmoid(s*z + b) + c,  A,s,b,c fitted
    The softmax ratio is invariant to the global scale A/2, so the kernel
    computes p = m * (tanh(SC*z + BH) + C2) with SC=s/2, BH=b/2,
    C2 = 1 + 2c/A: ONE tanh pass on ACT (bias slot carries SC*f1_j, so z
    is never materialized) and ONE fused tensor_scalar on DVE. The mask
    ships as additive {0,-10} fp8 and is applied BY THE DMA ENGINES
    (SWDGE accumulate-add into the tanh tile); p = max(t + C2, 0) then
    zeroes masked entries exactly. No exp pass, no mask multiply, no
    mask SBUF tile.
  * Row-sharding: each core owns N/8 output rows; softmax is row-wise so
    there is no cross-core reduction. Collectives measured ~90us of
    barrier+skew on this fabric, so instead of an AllGather each core
    recomputes X~ = nf @ [W | W@v0 | W@v1] for ALL rows from a replicated
    bf16 nf^T, pipelined in 128-row blocks so the attention pipeline
    starts as soon as block 0 lands.
  * The graph ships as fp8 {0,1} (4x less HBM traffic than int32) and is
    SWDGE-cast to bf16 on load; node_features/wext ship as bf16 (the
    matmuls run bf16 anyway, so no extra rounding).
"""

import numpy as np

import concourse.bass as bass
import concourse.mybir as mybir
import concourse.tile as tile
from concourse.bass_utils import run_bass_kernel_spmd

# ----------------------------------------------------------------------------
# Workaround for "Too many sync wait commands": this walrus build accepts only
# ONE sync-wait per instruction. Post-pass: hoist surplus waits onto
# single-wait NOPs on the same engine, inserted immediately before the
# instruction (identical blocking semantics, per-engine order preserved).
# ----------------------------------------------------------------------------


def _split_multi_waits(nc):
    import bass_rust

    eng = {
        mybir.EngineType.PE: nc.tensor,
        mybir.EngineType.DVE: nc.vector,
        mybir.EngineType.Activation: nc.scalar,
        mybir.EngineType.Pool: nc.gpsimd,
        mybir.EngineType.SP: nc.sync,
    }
    for f in nc.m.functions:
        for blk in f.blocks:
            fixups = []  # (index, inst, waits)
            for idx, inst in enumerate(blk.instructions):
                si = inst.sync_info
                waits = list(si.on_wait) if si is not None and si.on_wait else []
                if len(waits) > 1 and inst.engine in eng:
                    fixups.append((idx, inst, waits))
            if not fixups:
                continue
            nops_by_idx = {}
            created = set()
            for idx, inst, waits in fixups:
                inst.sync_info.on_wait = [waits[-1]]
                nops = []
                for w in waits[:-1]:
                    nop = eng[inst.engine].nop(nofuse=True, hint="wait_split").ins
                    nop.sync_info = bass_rust.SyncInfo(on_wait=[w], on_update=[])
                    nops.append(nop)
                    created.add(id(nop))
                nops_by_idx[idx] = nops
            # Drop the freshly-created nops from wherever nop() appended
            # them, then splice them in before their instruction.
            for b2 in f.blocks:
                b2.instructions[:] = [
                    i for i in b2.instructions if id(i) not in created
                ]
            new = []
            for idx, inst in enumerate(blk.instructions):
                new.extend(nops_by_idx.get(idx, ()))
                new.append(inst)
            blk.instructions[:] = new


# ----------------------------------------------------------------------------

F32 = mybir.dt.float32
BF16 = mybir.dt.bfloat16
FP8 = mybir.dt.float8e4
AF = mybir.ActivationFunctionType
ALU = mybir.AluOpType

N, D_IN, D_OUT = 8192, 512, 256
M_CORES = 8
P = 128

# affine-of-logistic fit of w*(z) = exp(sigmoid(z) - 0.5):
#   w* ~= A*sigmoid(s*z+b) + c, max abs err 5.3e-4 over z in [-14, 14]
_A, _S, _B, _C = 1.0418747, 1.01663968, -0.49796181, 0.60632424
SC = _S / 2.0            # tanh scale on z
BH = _B / 2.0            # tanh bias constant
C2 = 1.0 + 2.0 * _C / _A  # additive constant, global A/2 scale cancels


def build_gat(n=N, d_in=D_IN, d_out=D_OUT, m_cores=M_CORES, grp=4):
    """Per-core SPMD program. Inputs per core:
      graph_T  [n, R] fp8    -- additive mask: 0 edge / -10 no-edge, transposed
      nfT_mine [d_in, R] bf16  -- node_features[rows].T
      nfT_full [d_in, n] bf16  -- node_features.T (replicated)
      wext     [d_in, d_out+2] bf16 -- [W | W@v0 | W@v1] (replicated)
    Output: out [R, d_out] f32 (this core's rows)."""
    R = n // m_cores            # rows per core
    NJ = n // P                 # 128-wide j chunks over the full N
    IB = R // P                 # 128-row output blocks per core
    DK = d_in // P              # 128-deep contraction chunks
    DEXT = d_out + 2            # X | f0 | f1
    DW = d_out + 1              # main matmul rhs width: X | ones
    n_grp = NJ // grp

    nc = bass.Bass(num_devices=m_cores)
    g_t = nc.declare_dram_parameter("graph_T", [n, R], FP8, isOutput=False)
    nfT_mine = nc.declare_dram_parameter("nfT_mine", [d_in, R], BF16, isOutput=False)
    nfT_full = nc.declare_dram_parameter("nfT_full", [d_in, n], BF16, isOutput=False)
    wext = nc.declare_dram_parameter("wext", [d_in, DEXT], BF16, isOutput=False)
    outp = nc.declare_dram_parameter("out", [R, d_out], F32, isOutput=True)

    with tile.TileContext(nc) as tc:
        with tc.tile_pool(name="persist", bufs=1) as persist, \
             tc.tile_pool(name="dram", bufs=1, space="DRAM") as dram, \
             tc.tile_pool(name="psum", bufs=1, space="PSUM") as psb, \
             tc.tile_pool(name="nfc", bufs=5) as nf_pool, \
             tc.tile_pool(name="tg", bufs=6) as t_pool, \
             tc.tile_pool(name="phi", bufs=5) as phi_pool, \
             tc.tile_pool(name="plo", bufs=16) as plo_pool, \
             tc.tile_pool(name="epi", bufs=2) as epi:

            # The 8 PSUM banks triple-duty: f0 mini-matmuls, X~ block
            # accumulation, then the 64-chunk attention accumulation.
            psum = [
                psb.tile([P, DEXT], F32, tag=f"ps{ib}", name=f"psum{ib}")
                for ib in range(IB)
            ]

            xsb = persist.tile([P, NJ, DEXT], BF16)      # X~ all rows (bf16)
            f0rep = persist.tile([P, R], F32)            # f0 row, replicated
            fbias = persist.tile([P, NJ], F32)           # SC*f1 + BH per partition
            wextb = persist.tile([P, DK, DEXT], BF16)

            # bf16 nfm (f0 errors cancel in the softmax row-normalization);
            # FIRST on the sync queue — the whole tanh stream hangs off the
            # f0 path, so its input must land as early as possible
            nfm = persist.tile([P, DK, R], BF16)
            for kc in range(DK):
                nc.sync.dma_start(
                    out=nfm[:, kc, :],
                    in_=bass.AP(nfT_mine, kc * P * R, [[R, P], [1, R]]),
                )
            nc.sync.dma_start(
                out=wextb,
                in_=bass.AP(wext, 0, [[DEXT, P], [P * DEXT, DK], [1, DEXT]]),
            )

            # ---- f0 for own rows, computed in ROW form ------------------
            # out[1, i] = sum_d wv0[d] nf[i, d]: stationary = wv0 (M=1), so
            # f0 lands as a row directly — no transpose, no column copies.
            # prewarm the exp_and_others ACT table during the preamble
            warm = persist.tile([P, 1], F32)
            nc.gpsimd.memset(warm, 0.0)
            nc.scalar.activation(out=warm, in_=warm, func=AF.Tanh)

            wv0b = persist.tile([P, DK, 1], BF16)
            nc.scalar.dma_start(
                out=wv0b,
                in_=bass.AP(wext, d_out, [[DEXT, P], [P * DEXT, DK], [1, 1]]),
            )
            # f0 mini-matmuls live in banks 4-7 so the X~ stream (banks 0-1)
            # starts as soon as nfc lands, without waiting for the f0 path
            f0flat = persist.tile([1, R], F32)
            FN = 256 if (DEXT >= 256 and R % 256 == 0) else P
            for kc in range(DK):
                for sp in range(R // FN):
                    nc.tensor.matmul(
                        out=psum[4 + sp][0:1, 0:FN],
                        lhsT=wv0b[:, kc, :],
                        rhs=nfm[:, kc, sp * FN:(sp + 1) * FN],
                        start=(kc == 0),
                        stop=(kc == DK - 1),
                    )
            for sp in range(R // FN):
                nc.vector.tensor_copy(
                    out=f0flat[:, sp * FN:(sp + 1) * FN], in_=psum[4 + sp][0:1, 0:FN]
                )
            # broadcast across partitions via a DRAM bounce (SBUF-source
            # APs cannot have a zero partition step); on the scalar queue,
            # which is idle here — the sync queue is busy streaming nfc
            f0dram = dram.tile([R], F32)
            nc.scalar.dma_start(
                out=f0dram.rearrange("(o r) -> o r", o=1), in_=f0flat
            )
            nc.scalar.dma_start(
                out=f0rep, in_=bass.AP(f0dram.tensor, 0, [[0, P], [1, R]])
            )

            # ---- X~ blocks for ALL rows (streamed, replicated compute) ---
            NFG = grp  # X~ blocks per nf load == mask group size
            t_list = []
            for gb in range(NJ // NFG):
                nfc = nf_pool.tile([P, DK, NFG * P], BF16, tag="nfc", bufs=4)
                # NOTE: keep these off nc.scalar — scalar-queue DMAs execute
                # in the ACT engine's instruction stream and head-of-line
                # block the tanh pipeline on pool-slot waits. And keep them
                # off nc.gpsimd: the Pool SWDGE queue is dedicated to the
                # mask accumulate-DMAs, which sit on the per-group critical
                # chain.
                nc.sync.dma_start(
                    out=nfc,
                    in_=bass.AP(
                        nfT_full, gb * NFG * P,
                        [[n, P], [P * n, DK], [1, NFG * P]],
                    ),
                )
                for bb in range(NFG):
                    ib = gb * NFG + bb
                    # X~ ping-pongs on PSUM banks 0-1 only; banks 2-7 belong
                    # to the attention accumulators for row-blocks 2-7, which
                    # start accumulating while X~ is still streaming
                    ps = psum[ib % 2]
                    for kc in range(DK):
                        nc.tensor.matmul(
                            out=ps,
                            lhsT=nfc[:, kc, bb * P:(bb + 1) * P],
                            rhs=wextb[:, kc, :],
                            start=(kc == 0),
                            stop=(kc == DK - 1),
                        )
                    nc.vector.tensor_copy(out=xsb[:, ib, 0:DEXT], in_=ps)
                # batched per nf-group: tanh bias slice
                nc.vector.tensor_scalar(
                    out=fbias[:, gb * NFG:(gb + 1) * NFG],
                    in0=xsb[:, gb * NFG:(gb + 1) * NFG, d_out + 1],
                    scalar1=SC, scalar2=BH,
                    op0=ALU.mult, op1=ALU.add,
                )
                nc.vector.memset(
                    xsb[:, gb * NFG:(gb + 1) * NFG, d_out], 1.0
                )
                # ---- emit mask-group gb's ACT work here (NFG == grp): the
                # PSUM banks only constrain the matmuls, so the tanh pass can
                # run during the X~ phase, throttled by the t-pool slots.
                g = gb
                t_g = t_pool.tile([P, grp, R], BF16)
                for jj in range(grp):
                    jc = g * grp + jj
                    nc.scalar.activation(
                        out=t_g[:, jj, :],
                        in_=f0rep,
                        func=AF.Tanh,
                        bias=fbias[:, jc:jc + 1],
                        scale=SC,
                    )
                # mask applied by the DMA engines: graph_T ships as
                # {0, -10} fp8 and is accumulate-ADDed into the tanh tile
                # (SWDGE alu). Masked-off entries drop to ~-8, which the
                # relu in the next TS snaps to exactly 0.
                nc.gpsimd.dma_start(
                    out=t_g,
                    in_=bass.AP(g_t, g * grp * P * R, [[R, P], [P * R, grp], [1, R]]),
                    accum_op=ALU.add,
                )
                # p produced HERE, split by output row-block: p_hi (rows of
                # blocks 2-7) is consumed by attention matmuls interleaved
                # into this very loop (PSUM banks 2-7 are free — X~ only
                # ping-pongs banks 0-1), so its pool recycles mid-phase;
                # p_lo (rows of blocks 0-1) is tiny and all 16 groups stay
                # live for the post-X~ tail. No slot-wait cycles.
                RL = 2 * P                      # rows covered by p_lo
                p_hi = phi_pool.tile([P, grp, R - RL], BF16)
                nc.vector.tensor_scalar(
                    out=p_hi,
                    in0=t_g[:, :, RL:R],
                    scalar1=C2, scalar2=0.0,
                    op0=ALU.add, op1=ALU.max,
                )
                p_lo = plo_pool.tile([P, grp, RL], BF16)
                nc.vector.tensor_scalar(
                    out=p_lo,
                    in0=t_g[:, :, 0:RL],
                    scalar1=C2, scalar2=0.0,
                    op0=ALU.add, op1=ALU.max,
                )
                t_list.append((p_hi, p_lo))

                # attention for row-blocks 2-7, LAG groups behind production
                LAG = 2
                if gb >= LAG:
                    g_att = gb - LAG
                    ph = t_list[g_att][0]
                    for jj in range(grp):
                        jc = g_att * grp + jj
                        for ib in range(2, IB):
                            nc.tensor.matmul(
                                out=psum[ib][:, 0:DW],
                                lhsT=ph[:, jj, (ib - 2) * P:(ib - 1) * P],
                                rhs=xsb[:, jc, 0:DW],
                                start=(jc == 0),
                                stop=(jc == NJ - 1),
                            )

            # ---- row-blocks 0-1 for all groups (banks 0-1 free post-X~;
            # most p_lo tiles are long since ready, so this PE stretch runs
            # while the last tanh groups are still streaming) -------------
            for g in range(n_grp):
                pl = t_list[g][1]
                for jj in range(grp):
                    jc = g * grp + jj
                    for ib in range(2):
                        nc.tensor.matmul(
                            out=psum[ib][:, 0:DW],
                            lhsT=pl[:, jj, ib * P:(ib + 1) * P],
                            rhs=xsb[:, jc, 0:DW],
                            start=(jc == 0),
                            stop=(jc == NJ - 1),
                        )
            # ---- attention stragglers (last LAG groups, row-blocks 2-7) --
            for g_att in range(n_grp - 2, n_grp):
                ph = t_list[g_att][0]
                for jj in range(grp):
                    jc = g_att * grp + jj
                    for ib in range(2, IB):
                        nc.tensor.matmul(
                            out=psum[ib][:, 0:DW],
                            lhsT=ph[:, jj, (ib - 2) * P:(ib - 1) * P],
                            rhs=xsb[:, jc, 0:DW],
                            start=(jc == 0),
                            stop=(jc == NJ - 1),
                        )

            # ---- epilogue: normalize + elu + store -----------------------
            for ib in (0, 1, 2, 3, 4, 5, 6, 7):
                o = epi.tile([P, DW], F32, tag="o")
                # ScE sits closer to PSUM and is idle in the tail
                nc.scalar.copy(out=o, in_=psum[ib][:, 0:DW])
                sm = epi.tile([P, 1], F32, tag="sm")
                nc.vector.tensor_scalar_max(sm, o[:, d_out:DW], 1e-30)
                r = epi.tile([P, 1], F32, tag="r")
                nc.vector.reciprocal(out=r, in_=sm)
                u = epi.tile([P, d_out], F32, tag="u")
                nc.vector.tensor_scalar(
                    out=u, in0=o[:, 0:d_out], scalar1=r, scalar2=None,
                    op0=ALU.mult,
                )
                rp = epi.tile([P, d_out], F32, tag="rp")
                # max(u,0) - 1 fused in one tensor_scalar
                nc.vector.tensor_scalar(
                    out=rp, in0=u, scalar1=0.0, scalar2=-1.0,
                    op0=ALU.max, op1=ALU.add,
                )
                xm = epi.tile([P, d_out], F32, tag="xm")
                nc.vector.tensor_scalar_min(xm, u, 0.0)
                en = epi.tile([P, d_out], F32, tag="en")
                nc.scalar.activation(out=en, in_=xm, func=AF.Exp)
                res = epi.tile([P, d_out], F32, tag="res")
                nc.vector.tensor_tensor(out=res, in0=en, in1=rp, op=ALU.add)
                nc.sync.dma_start(out=outp[ib * P:(ib + 1) * P, :], in_=res)

    _split_multi_waits(nc)
    return nc


_cached = {}

# Dev/test knobs (the grading harness just calls kernel(**inputs)):
_TRACE = False
_TMPDIR = None
_LAST_EXEC_NS = None
_LAST_RESULTS = None


def _get_program(n, d_in, d_out, m_cores):
    key = (n, d_in, d_out, m_cores)
    if key not in _cached:
        _cached[key] = build_gat(n, d_in, d_out, m_cores)
    return _cached[key]


def kernel(node_features, graph, W, v0, v1):
    import ml_dtypes

    node_features = np.asarray(node_features, dtype=np.float32)
    graph = np.asarray(graph)
    W = np.asarray(W, dtype=np.float32)
    v0 = np.asarray(v0, dtype=np.float32)
    v1 = np.asarray(v1, dtype=np.float32)

    n, d_in = node_features.shape
    d_out = W.shape[1]
    m = M_CORES
    R = n // m

    nc = _get_program(n, d_in, d_out, m)

    bf16 = ml_dtypes.bfloat16
    fp8 = ml_dtypes.float8_e4m3
    wext = np.concatenate([W, W @ v0, W @ v1], axis=1).astype(bf16)
    nfT_full = np.ascontiguousarray(node_features.T).astype(bf16)
    # additive mask: 0 where edge present, -10 where absent (relu'd out)
    mask8 = np.where(np.asarray(graph) != 0, 0.0, -10.0).astype(fp8)
    in_maps = []
    for c in range(m):
        rows = slice(c * R, (c + 1) * R)
        in_maps.append({
            "graph_T": np.ascontiguousarray(mask8[rows].T),
            "nfT_mine": np.ascontiguousarray(node_features[rows].T).astype(bf16),
            "nfT_full": nfT_full,
            "wext": wext,
        })
    global _LAST_EXEC_NS, _LAST_RESULTS
    res = run_bass_kernel_spmd(
        nc, in_maps, list(range(m)), trace=_TRACE, tmpdir=_TMPDIR
    )
    _LAST_EXEC_NS = res.exec_time_ns
    _LAST_RESULTS = res
    return np.concatenate([res.results[c]["out"] for c in range(m)], axis=0)
